# revision 3
# baseline (speedup 1.0000x reference)
"""Fixed-point (MPC) 3x3 VALID conv2d, NHWC, f32 — Trainium2 Bass kernel.

Semantics (bit-exact vs the jax reference, fixed_point=8, S=256):
    qx = round_half_even(x*S)/S ; qw = round_half_even(w*S)/S
    y  = conv2d_valid(qx, qw)   ; out = floor(y*S)/S

Everything is exact integer arithmetic in disguise:
  x_int = RNE(x*256) fits fp16 exactly (|x_int| < 2048), w_int fits fp16,
  products/partial sums stay < 2^24 so fp32 PSUM accumulation is exact,
  and floor() is done with exact float tricks (verified in test harness).

Strategy per core (data-parallel over batch, 4 images/core):
  - load x rows contiguously: [h, (w,c)] f32 tiles (perfect DMA)
  - quantize: t = x*256 + 1.5*2^23 (ACT), x_int16 = t - 1.5*2^23 -> fp16 (DVE)
  - PE-transpose 8-w-wide windows to [(8w,16c), h] fp16 layout
  - banded-weight matmul: lhsT[128, 96] per kh tap; partitions = (dw,c),
    columns = (out_w j in 0..5, out_ch k); 3 h-shifted matmuls accumulate
    in PSUM => 6 output columns x 16 ch x 444 pixels per 3 matmuls
  - floor: v1 = y_int/256 - 255/512 (ACT), v2 = v1 + 1.5*2^23 (DVE, RNE
    leaves floor+C), PE-transpose back to [h', (w',k)], final
    out = v2/256 - 49152 fused into the PSUM->SBUF copy (DVE)
  - store [h', 3552] f32 rows contiguously (perfect DMA)
"""

import numpy as np

import concourse.mybir as mybir
from concourse import bass, tile
from concourse.masks import make_identity

N_CORES = 8
B_FULL = 32
B_CORE = B_FULL // N_CORES  # 4 images per core
H = W = 224
C = K = 16
HO = WO = 222

F32 = mybir.dt.float32
F16 = mybir.dt.float16

C_RND = 12582912.0  # 1.5 * 2**23: magic addend, RNE-to-integer for |v| < 2**22
INV_S = 1.0 / 256.0
FLOOR_BIAS = -255.0 / 512.0
OUT_BIAS = -49152.0  # -(C_RND / 256)

N_BLK = 37  # 37 blocks x 6 output w's = 222


def _split_multi_waits(nc):
    """The installed walrus only encodes ONE sync wait per instruction.
    Hoist extra waits onto NoOps inserted just before, same engine."""
    for f in nc.m.functions:
        for bb in f.blocks:
            new_list = []
            changed = False
            for ins in bb.instructions:
                si = ins.sync_info
                if si is not None and si.on_wait and len(si.on_wait) > 1:
                    waits = list(si.on_wait)
                    for wt in waits[:-1]:
                        nop = mybir.InstNoOp(
                            name=f"NOPW-{nc.next_id()}", ins=[], outs=[]
                        )
                        nop.engine = ins.engine
                        nop.sync_info = mybir.SyncInfo(on_wait=[wt], on_update=[])
                        new_list.append(nop)
                    ins.sync_info = mybir.SyncInfo(
                        on_wait=[waits[-1]], on_update=list(si.on_update or [])
                    )
                    changed = True
                new_list.append(ins)
            if changed:
                bb.instructions = new_list


def _build_nc():
    nc = bass.Bass("TRN2", num_devices=N_CORES)
    x_d = nc.dram_tensor("x", [B_CORE * H, W * C], F32, kind="ExternalInput")
    wb_d = nc.dram_tensor("wb", [3, 128, 96], F16, kind="ExternalInput")
    y_d = nc.dram_tensor("y", [B_CORE, HO, WO * K], F32, kind="ExternalOutput")

    add = mybir.AluOpType.add
    mult = mybir.AluOpType.mult
    COPY = mybir.ActivationFunctionType.Copy

    htiles = ((0, 128), (128, 96))  # (row offset, rows) covering 224
    hchunks = ((0, 128), (128, 94))  # output h' chunks covering 222

    with tile.TileContext(nc) as tc:
        with (
            tc.tile_pool(name="consts", bufs=1) as consts,
            tc.tile_pool(name="xf", bufs=2) as xf_pool,
            tc.tile_pool(name="xq", bufs=1) as xq_pool,
            tc.tile_pool(name="xqt", bufs=4) as xqt_pool,
            tc.tile_pool(name="v", bufs=3) as v_pool,
            tc.tile_pool(name="st", bufs=1) as st_pool,
            tc.tile_pool(name="pst", bufs=2, space="PSUM") as ps_in_pool,
            tc.tile_pool(name="psy", bufs=2, space="PSUM") as ps_y_pool,
            tc.tile_pool(name="pso", bufs=2, space="PSUM") as ps_o_pool,
        ):
            ident16 = consts.tile([128, 128], F16, tag="id16")
            make_identity(nc, ident16[:])
            ident32 = consts.tile([128, 128], F32, tag="id32")
            make_identity(nc, ident32[:])
            wtiles = []
            for kh in range(3):
                wt = consts.tile([128, 96], F16, tag=f"w{kh}")
                nc.sync.dma_start(out=wt[:], in_=wb_d[kh])
                wtiles.append(wt)

            for pair in range(2):
                # ---- load + quantize both images of the pair ----
                xq_tiles = {}
                for ii in range(2):
                    img = 2 * pair + ii
                    for ht, (r0, pr) in enumerate(htiles):
                        xf = xf_pool.tile([128, W * C], F32, tag="xf")
                        nc.sync.dma_start(
                            out=xf[:pr, :],
                            in_=x_d[H * img + r0 : H * img + r0 + pr, :],
                        )
                        tt = xf_pool.tile([128, W * C], F32, tag="tt")
                        nc.scalar.activation(
                            out=tt[:pr, :], in_=xf[:pr, :], func=COPY,
                            bias=C_RND, scale=256.0,
                        )
                        xq = xq_pool.tile([128, W * C], F16, tag=f"xq{ii}{ht}")
                        nc.vector.tensor_scalar(
                            out=xq[:pr, :], in0=tt[:pr, :],
                            scalar1=-C_RND, scalar2=None, op0=add,
                        )
                        xq_tiles[(ii, ht)] = xq

                st_tiles = {}
                for ii in range(2):
                    for ch, (h0, hc) in enumerate(hchunks):
                        st_tiles[(ii, ch)] = st_pool.tile(
                            [128, N_BLK * 96], F32, tag=f"st{ii}{ch}",
                            name=f"st{ii}{ch}",
                        )

                # ---- per w-block: transpose in, conv, floor, transpose out ----
                for blk in range(N_BLK):
                    xqt = xqt_pool.tile([128, 2, 224], F16, tag="xqt")
                    for ii in range(2):
                        for ht, (r0, pr) in enumerate(htiles):
                            pst = ps_in_pool.tile([128, 128], F16, tag="pst")
                            nc.tensor.transpose(
                                out=pst[:, :pr],
                                in_=xq_tiles[(ii, ht)][:pr, 96 * blk : 96 * blk + 128],
                                identity=ident16[:pr, :pr],
                            )
                            if ht == 0:
                                nc.vector.tensor_copy(
                                    out=xqt[:, ii, r0 : r0 + pr], in_=pst[:, :pr]
                                )
                            else:
                                nc.scalar.activation(
                                    out=xqt[:, ii, r0 : r0 + pr], in_=pst[:, :pr],
                                    func=COPY,
                                )

                    psy = ps_y_pool.tile([96, 2, WO], F32, tag="psy")
                    for s in range(3):
                        nc.tensor.matmul(
                            out=psy[:],
                            lhsT=wtiles[s][:],
                            rhs=xqt[:, :, s : s + WO],
                            start=(s == 0),
                            stop=(s == 2),
                        )

                    v1 = v_pool.tile([96, 2, WO], F32, tag="v1")
                    nc.scalar.activation(
                        out=v1[:], in_=psy[:], func=COPY,
                        bias=FLOOR_BIAS, scale=INV_S,
                    )
                    v2 = v_pool.tile([96, 2, WO], F32, tag="v2")
                    nc.vector.tensor_scalar(
                        out=v2[:], in0=v1[:], scalar1=C_RND, scalar2=None, op0=add,
                    )

                    for ii in range(2):
                        for ch, (h0, hc) in enumerate(hchunks):
                            pso = ps_o_pool.tile([128, 96], F32, tag="pso")
                            nc.tensor.transpose(
                                out=pso[:hc, :],
                                in_=v2[:, ii, h0 : h0 + hc],
                                identity=ident32[:96, :96],
                            )
                            nc.vector.tensor_scalar(
                                out=st_tiles[(ii, ch)][:hc, 96 * blk : 96 * blk + 96],
                                in0=pso[:hc, :],
                                scalar1=INV_S, scalar2=OUT_BIAS,
                                op0=mult, op1=add,
                            )

                # ---- store ----
                for ii in range(2):
                    img = 2 * pair + ii
                    for ch, (h0, hc) in enumerate(hchunks):
                        nc.sync.dma_start(
                            out=y_d[img, h0 : h0 + hc, :],
                            in_=st_tiles[(ii, ch)][:hc, :],
                        )

    _split_multi_waits(nc)
    return nc


def _banded_weights(w: np.ndarray) -> np.ndarray:
    """w [3,3,16,16] f32 -> wb [3, 128, 96] fp16 banded lhsT matrices.

    wb[kh][16*dw + c, 16*j + k] = round(w*256)[kh, dw - j, c, k]
    for 0 <= dw - j <= 2, j in 0..5."""
    wq = np.round(w.astype(np.float32) * np.float32(256.0))  # RNE, exact
    assert np.abs(wq).max() < 240, "w_int exceeds fp16-exact budget"
    wb = np.zeros((3, 128, 96), dtype=np.float32)
    for kh in range(3):
        for j in range(6):
            for kw in range(3):
                dw = j + kw
                wb[kh, 16 * dw : 16 * dw + 16, 16 * j : 16 * j + 16] = wq[kh, kw]
    return wb.astype(np.float16)


_RUNNER = None


def _get_runner():
    """Build the Bass program once and return a callable(in_maps)->results.

    Mirrors concourse.bass2jax.run_bass_via_pjrt's multi-core path but
    caches the jitted executable so repeated calls don't recompile."""
    global _RUNNER
    if _RUNNER is not None:
        return _RUNNER

    import jax
    from jax.sharding import Mesh, PartitionSpec
    from jax.experimental.shard_map import shard_map
    from concourse.bass2jax import (
        _bass_exec_p,
        install_neuronx_cc_hook,
        partition_id_tensor,
    )

    install_neuronx_cc_hook()
    nc = _build_nc()

    partition_name = nc.partition_id_tensor.name if nc.partition_id_tensor else None
    in_names, out_names, out_avals, zero_outs = [], [], [], []
    for alloc in nc.m.functions[0].allocations:
        if not isinstance(alloc, mybir.MemoryLocationSet):
            continue
        name = alloc.memorylocations[0].name
        if alloc.kind == "ExternalInput":
            if name != partition_name:
                in_names.append(name)
        elif alloc.kind == "ExternalOutput":
            out_names.append(name)
            shape = tuple(alloc.tensor_shape)
            dtype = mybir.dt.np(alloc.dtype)
            out_avals.append(jax.core.ShapedArray(shape, dtype))
            zero_outs.append(np.zeros(shape, dtype))
    n_params = len(in_names)
    n_outs = len(out_avals)
    all_in_names = list(in_names) + list(out_names)
    if partition_name is not None:
        all_in_names.append(partition_name)

    def _body(*args):
        operands = list(args)
        if partition_name is not None:
            operands.append(partition_id_tensor())
        outs = _bass_exec_p.bind(
            *operands,
            out_avals=tuple(out_avals),
            in_names=tuple(all_in_names),
            out_names=tuple(out_names),
            lowering_input_output_aliases=(),
            sim_require_finite=True,
            sim_require_nnan=True,
            nc=nc,
        )
        return tuple(outs)

    devices = jax.devices()[:N_CORES]
    assert len(devices) == N_CORES, f"need {N_CORES} devices, got {len(devices)}"
    mesh = Mesh(np.asarray(devices), ("core",))
    in_specs = (PartitionSpec("core"),) * (n_params + n_outs)
    out_specs = (PartitionSpec("core"),) * n_outs
    sharded = jax.jit(
        shard_map(_body, mesh=mesh, in_specs=in_specs, out_specs=out_specs,
                  check_rep=False),
        donate_argnums=tuple(range(n_params, n_params + n_outs)),
        keep_unused=True,
    )

    state = {
        "sharded": sharded,
        "in_names": in_names,
        "out_names": out_names,
        "out_avals": out_avals,
        "zero_outs": zero_outs,
        "n_cores": N_CORES,
    }

    def runner(in_maps):
        per_core = [[np.asarray(m[nm]) for nm in in_names] for m in in_maps]
        concat_in = [
            np.concatenate([per_core[c][i] for c in range(N_CORES)], axis=0)
            for i in range(n_params)
        ]
        concat_zeros = [
            np.zeros((N_CORES * z.shape[0], *z.shape[1:]), z.dtype)
            for z in zero_outs
        ]
        out_arrs = state["sharded"](*concat_in, *concat_zeros)
        return [
            {
                nm: np.asarray(out_arrs[i]).reshape(
                    N_CORES, *out_avals[i].shape
                )[c]
                for i, nm in enumerate(out_names)
            }
            for c in range(N_CORES)
        ]

    runner.state = state
    _RUNNER = runner
    return _RUNNER


def kernel(x: np.ndarray, w: np.ndarray, fixed_point) -> np.ndarray:
    assert int(fixed_point) == 8, f"kernel hardcodes fixed_point=8, got {fixed_point}"
    x = np.ascontiguousarray(np.asarray(x, dtype=np.float32))
    assert x.shape == (B_FULL, H, W, C), x.shape
    assert np.abs(x).max() * 256.0 < 2040.0, "x_int exceeds fp16-exact budget"

    wb = _banded_weights(np.asarray(w, dtype=np.float32))
    runner = _get_runner()

    in_maps = []
    for core in range(N_CORES):
        xs = x[B_CORE * core : B_CORE * (core + 1)].reshape(B_CORE * H, W * C)
        in_maps.append({"x": xs, "wb": wb})

    results = runner(in_maps)
    out = np.concatenate(
        [r["y"].reshape(B_CORE, HO, WO, K) for r in results], axis=0
    )
    return out


# revision 6
# speedup vs baseline: 51.3462x; 51.3462x over previous
"""Fixed-point (MPC) 3x3 VALID conv2d, NHWC, f32 — Trainium2 Bass kernel.

Semantics (bit-exact vs the jax reference, fixed_point=8, S=256):
    qx = round_half_even(x*S)/S ; qw = round_half_even(w*S)/S
    y  = conv2d_valid(qx, qw)   ; out = floor(y*S)/S

Everything is exact integer arithmetic in disguise:
  x_int = RNE(x*256) fits fp16 exactly (|x_int| < 2048), w_int fits fp16,
  products/partial sums stay < 2^24 so fp32 PSUM accumulation is exact,
  and floor() is done with exact float tricks (verified in test harness).

Strategy per core (data-parallel over batch, 4 images/core):
  - load x rows contiguously: [h, (w,c)] f32 tiles (perfect DMA)
  - quantize: t = x*256 + 1.5*2^23 (ACT), x_int16 = t - 1.5*2^23 -> fp16 (DVE)
  - PE-transpose 8-w-wide windows to [(8w,16c), h] fp16 layout
  - banded-weight matmul: lhsT[128, 96] per kh tap; partitions = (dw,c),
    columns = (out_w j in 0..5, out_ch k); 3 h-shifted matmuls accumulate
    in PSUM => 6 output columns x 16 ch x 444 pixels per 3 matmuls
  - floor: v1 = y_int/256 - 255/512 (ACT), v2 = v1 + 1.5*2^23 (DVE, RNE
    leaves floor+C), PE-transpose back to [h', (w',k)], final
    out = v2/256 - 49152 fused into the PSUM->SBUF copy (DVE)
  - store [h', 3552] f32 rows contiguously (perfect DMA)
"""

import numpy as np

import concourse.mybir as mybir
from concourse import bass, tile
from concourse.masks import make_identity

N_CORES = 8
B_FULL = 32
B_CORE = B_FULL // N_CORES  # 4 images per core
H = W = 224
C = K = 16
HO = WO = 222

F32 = mybir.dt.float32
F16 = mybir.dt.float16

C_RND = 12582912.0  # 1.5 * 2**23: magic addend, RNE-to-integer for |v| < 2**22
INV_S = 1.0 / 256.0
FLOOR_BIAS = -255.0 / 512.0
OUT_BIAS = -49152.0  # -(C_RND / 256)

N_BLK = 37  # 37 blocks x 6 output w's = 222


def _split_multi_waits(nc):
    """The installed walrus only encodes ONE sync wait per instruction.
    Hoist extra waits onto NoOps inserted just before, same engine."""
    for f in nc.m.functions:
        for bb in f.blocks:
            new_list = []
            changed = False
            for ins in bb.instructions:
                si = ins.sync_info
                if si is not None and si.on_wait and len(si.on_wait) > 1:
                    waits = list(si.on_wait)
                    for wt in waits[:-1]:
                        nop = mybir.InstNoOp(
                            name=f"NOPW-{nc.next_id()}", ins=[], outs=[]
                        )
                        nop.engine = ins.engine
                        nop.sync_info = mybir.SyncInfo(on_wait=[wt], on_update=[])
                        new_list.append(nop)
                    ins.sync_info = mybir.SyncInfo(
                        on_wait=[waits[-1]], on_update=list(si.on_update or [])
                    )
                    changed = True
                new_list.append(ins)
            if changed:
                bb.instructions = new_list


def _build_nc(stage_limit: int = 7):
    # stage_limit: 1=loads 2=+quant 3=+in_transpose 4=+conv 5=+floor
    #              6=+out_transpose 7=+store (full kernel). For ablation
    #              profiling with TimelineSim only; the runner uses 7.
    nc = bass.Bass("TRN2", num_devices=N_CORES)
    x_d = nc.dram_tensor("x", [B_CORE * H, W * C], F32, kind="ExternalInput")
    wb_d = nc.dram_tensor("wb", [3, 128, 96], F16, kind="ExternalInput")
    y_d = nc.dram_tensor("y", [B_CORE, HO, WO * K], F32, kind="ExternalOutput")

    add = mybir.AluOpType.add
    mult = mybir.AluOpType.mult
    COPY = mybir.ActivationFunctionType.Copy

    htiles = ((0, 128), (128, 96))  # (row offset, rows) covering 224
    hchunks = ((0, 128), (128, 94))  # output h' chunks covering 222

    with tile.TileContext(nc) as tc:
        with (
            tc.tile_pool(name="consts", bufs=1) as consts,
            tc.tile_pool(name="xf", bufs=2) as xf_pool,
            tc.tile_pool(name="xq", bufs=1) as xq_pool,
            tc.tile_pool(name="xqt", bufs=4) as xqt_pool,
            tc.tile_pool(name="v", bufs=3) as v_pool,
            tc.tile_pool(name="st", bufs=1) as st_pool,
            tc.tile_pool(name="pst", bufs=3, space="PSUM") as ps_in_pool,
            tc.tile_pool(name="psy", bufs=2, space="PSUM") as ps_y_pool,
            tc.tile_pool(name="pso", bufs=3, space="PSUM") as ps_o_pool,
        ):
            ident16 = consts.tile([128, 128], F16, tag="id16")
            make_identity(nc, ident16[:])
            ident32 = consts.tile([128, 128], F32, tag="id32")
            make_identity(nc, ident32[:])
            wtiles = []
            for kh in range(3):
                wt = consts.tile([128, 96], F16, tag=f"w{kh}")
                nc.sync.dma_start(out=wt[:], in_=wb_d[kh])
                wtiles.append(wt)

            for pair in range(2):
                # ---- load + quantize both images of the pair ----
                xq_tiles = {}
                for ii in range(2):
                    img = 2 * pair + ii
                    for ht, (r0, pr) in enumerate(htiles):
                        xf = xf_pool.tile([128, W * C], F32, tag="xf")
                        nc.sync.dma_start(
                            out=xf[:pr, :],
                            in_=x_d[H * img + r0 : H * img + r0 + pr, :],
                        )
                        if stage_limit < 2:
                            continue
                        tt = xf_pool.tile([128, W * C], F32, tag="tt")
                        nc.scalar.activation(
                            out=tt[:pr, :], in_=xf[:pr, :], func=COPY,
                            bias=C_RND, scale=256.0,
                        )
                        xq = xq_pool.tile([128, W * C], F16, tag=f"xq{ii}{ht}")
                        nc.vector.tensor_scalar(
                            out=xq[:pr, :], in0=tt[:pr, :],
                            scalar1=-C_RND, scalar2=None, op0=add,
                        )
                        xq_tiles[(ii, ht)] = xq

                st_all = st_pool.tile(
                    [128, 2, 2, N_BLK * 96], F32, tag="st_all", name="st_all"
                )

                # ---- per w-block: transpose in, conv, floor, transpose out ----
                for blk in range(N_BLK):
                    if stage_limit < 3:
                        break
                    xqt = xqt_pool.tile([128, 2, 224], F16, tag="xqt")
                    pst = ps_in_pool.tile([128, 2, 224], F16, tag="pst")
                    for ii in range(2):
                        for ht, (r0, pr) in enumerate(htiles):
                            nc.tensor.transpose(
                                out=pst[:, ii, r0 : r0 + pr],
                                in_=xq_tiles[(ii, ht)][:pr, 96 * blk : 96 * blk + 128],
                                identity=ident16[:pr, :pr],
                            )
                    if blk % 2 == 0:
                        nc.vector.tensor_copy(out=xqt[:], in_=pst[:])
                    else:
                        nc.scalar.activation(out=xqt[:], in_=pst[:], func=COPY)

                    if stage_limit < 4:
                        continue
                    psy = ps_y_pool.tile([96, 2, WO], F32, tag="psy")
                    for s in range(3):
                        nc.tensor.matmul(
                            out=psy[:],
                            lhsT=wtiles[s][:],
                            rhs=xqt[:, :, s : s + WO],
                            start=(s == 0),
                            stop=(s == 2),
                        )

                    if stage_limit < 5:
                        continue
                    v1 = v_pool.tile([96, 2, WO], F32, tag="v1")
                    nc.scalar.activation(
                        out=v1[:], in_=psy[:], func=COPY,
                        bias=FLOOR_BIAS, scale=INV_S,
                    )
                    v2 = v_pool.tile([96, 2, WO], F32, tag="v2")
                    nc.vector.tensor_scalar(
                        out=v2[:], in0=v1[:], scalar1=C_RND, scalar2=None, op0=add,
                    )

                    if stage_limit < 6:
                        continue
                    pso = ps_o_pool.tile([128, 2, 2, 96], F32, tag="pso")
                    for ii in range(2):
                        for ch, (h0, hc) in enumerate(hchunks):
                            nc.tensor.transpose(
                                out=pso[:hc, ii, ch, :],
                                in_=v2[:, ii, h0 : h0 + hc],
                                identity=ident32[:96, :96],
                            )
                    nc.vector.tensor_scalar(
                        out=st_all[:, :, :, 96 * blk : 96 * blk + 96],
                        in0=pso[:],
                        scalar1=INV_S, scalar2=OUT_BIAS,
                        op0=mult, op1=add,
                    )
                    if blk == 18 and stage_limit >= 7:
                        for ii in range(2):
                            img = 2 * pair + ii
                            for ch, (h0, hc) in enumerate(hchunks):
                                nc.sync.dma_start(
                                    out=y_d[img, h0 : h0 + hc, : 19 * 96],
                                    in_=st_all[:hc, ii, ch, : 19 * 96],
                                )

                # ---- store (blocks 19..36; 0..18 streamed at blk 18) ----
                if stage_limit < 7:
                    continue
                for ii in range(2):
                    img = 2 * pair + ii
                    for ch, (h0, hc) in enumerate(hchunks):
                        nc.sync.dma_start(
                            out=y_d[img, h0 : h0 + hc, 19 * 96 :],
                            in_=st_all[:hc, ii, ch, 19 * 96 :],
                        )

    _split_multi_waits(nc)
    return nc


def _banded_weights(w: np.ndarray) -> np.ndarray:
    """w [3,3,16,16] f32 -> wb [3, 128, 96] fp16 banded lhsT matrices.

    wb[kh][16*dw + c, 16*j + k] = round(w*256)[kh, dw - j, c, k]
    for 0 <= dw - j <= 2, j in 0..5."""
    wq = np.round(w.astype(np.float32) * np.float32(256.0))  # RNE, exact
    assert np.abs(wq).max() < 240, "w_int exceeds fp16-exact budget"
    wb = np.zeros((3, 128, 96), dtype=np.float32)
    for kh in range(3):
        for j in range(6):
            for kw in range(3):
                dw = j + kw
                wb[kh, 16 * dw : 16 * dw + 16, 16 * j : 16 * j + 16] = wq[kh, kw]
    return wb.astype(np.float16)


_RUNNER = None


def _get_runner():
    """Build the Bass program once and return a callable(in_maps)->results.

    Mirrors concourse.bass2jax.run_bass_via_pjrt's multi-core path but
    caches the jitted executable so repeated calls don't recompile."""
    global _RUNNER
    if _RUNNER is not None:
        return _RUNNER

    import jax
    from jax.sharding import Mesh, PartitionSpec
    from jax.experimental.shard_map import shard_map
    from concourse.bass2jax import (
        _bass_exec_p,
        install_neuronx_cc_hook,
        partition_id_tensor,
    )

    install_neuronx_cc_hook()
    nc = _build_nc()

    partition_name = nc.partition_id_tensor.name if nc.partition_id_tensor else None
    in_names, out_names, out_avals, zero_outs = [], [], [], []
    for alloc in nc.m.functions[0].allocations:
        if not isinstance(alloc, mybir.MemoryLocationSet):
            continue
        name = alloc.memorylocations[0].name
        if alloc.kind == "ExternalInput":
            if name != partition_name:
                in_names.append(name)
        elif alloc.kind == "ExternalOutput":
            out_names.append(name)
            shape = tuple(alloc.tensor_shape)
            dtype = mybir.dt.np(alloc.dtype)
            out_avals.append(jax.core.ShapedArray(shape, dtype))
            zero_outs.append(np.zeros(shape, dtype))
    n_params = len(in_names)
    n_outs = len(out_avals)
    all_in_names = list(in_names) + list(out_names)
    if partition_name is not None:
        all_in_names.append(partition_name)

    def _body(*args):
        operands = list(args)
        if partition_name is not None:
            operands.append(partition_id_tensor())
        outs = _bass_exec_p.bind(
            *operands,
            out_avals=tuple(out_avals),
            in_names=tuple(all_in_names),
            out_names=tuple(out_names),
            lowering_input_output_aliases=(),
            sim_require_finite=True,
            sim_require_nnan=True,
            nc=nc,
        )
        return tuple(outs)

    devices = jax.devices()[:N_CORES]
    assert len(devices) == N_CORES, f"need {N_CORES} devices, got {len(devices)}"
    mesh = Mesh(np.asarray(devices), ("core",))
    in_specs = (PartitionSpec("core"),) * (n_params + n_outs)
    out_specs = (PartitionSpec("core"),) * n_outs
    sharded = jax.jit(
        shard_map(_body, mesh=mesh, in_specs=in_specs, out_specs=out_specs,
                  check_rep=False),
        donate_argnums=tuple(range(n_params, n_params + n_outs)),
        keep_unused=True,
    )

    state = {
        "sharded": sharded,
        "in_names": in_names,
        "out_names": out_names,
        "out_avals": out_avals,
        "zero_outs": zero_outs,
        "n_cores": N_CORES,
    }

    def runner(in_maps):
        per_core = [[np.asarray(m[nm]) for nm in in_names] for m in in_maps]
        concat_in = [
            np.concatenate([per_core[c][i] for c in range(N_CORES)], axis=0)
            for i in range(n_params)
        ]
        concat_zeros = [
            np.zeros((N_CORES * z.shape[0], *z.shape[1:]), z.dtype)
            for z in zero_outs
        ]
        out_arrs = state["sharded"](*concat_in, *concat_zeros)
        return [
            {
                nm: np.asarray(out_arrs[i]).reshape(
                    N_CORES, *out_avals[i].shape
                )[c]
                for i, nm in enumerate(out_names)
            }
            for c in range(N_CORES)
        ]

    runner.state = state
    _RUNNER = runner
    return _RUNNER


def kernel(x: np.ndarray, w: np.ndarray, fixed_point) -> np.ndarray:
    assert int(fixed_point) == 8, f"kernel hardcodes fixed_point=8, got {fixed_point}"
    x = np.ascontiguousarray(np.asarray(x, dtype=np.float32))
    assert x.shape == (B_FULL, H, W, C), x.shape
    assert np.abs(x).max() * 256.0 < 2040.0, "x_int exceeds fp16-exact budget"

    wb = _banded_weights(np.asarray(w, dtype=np.float32))
    runner = _get_runner()

    in_maps = []
    for core in range(N_CORES):
        xs = x[B_CORE * core : B_CORE * (core + 1)].reshape(B_CORE * H, W * C)
        in_maps.append({"x": xs, "wb": wb})

    results = runner(in_maps)
    out = np.concatenate(
        [r["y"].reshape(B_CORE, HO, WO, K) for r in results], axis=0
    )
    return out


# revision 8
# speedup vs baseline: 30896.4217x; 601.7273x over previous
"""Fixed-point (MPC) 3x3 VALID conv2d, NHWC, f32 — Trainium2 Bass kernel.

Semantics (bit-exact vs the jax reference, fixed_point=8, S=256):
    qx = round_half_even(x*S)/S ; qw = round_half_even(w*S)/S
    y  = conv2d_valid(qx, qw)   ; out = floor(y*S)/S

Everything is exact integer arithmetic in disguise:
  x_int = RNE(x*256) fits fp16 exactly (|x_int| < 2048), w_int fits fp16,
  products/partial sums stay < 2^24 so fp32 PSUM accumulation is exact,
  and floor() is done with exact float tricks (verified in test harness).

Strategy per core (data-parallel over batch, 4 images/core):
  - load x rows contiguously: [h, (w,c)] f32 tiles (perfect DMA)
  - quantize: t = x*256 + 1.5*2^23 (ACT), x_int16 = t - 1.5*2^23 -> fp16 (DVE)
  - PE-transpose 8-w-wide windows to [(8w,16c), h] fp16 layout
  - banded-weight matmul: lhsT[128, 96] per kh tap; partitions = (dw,c),
    columns = (out_w j in 0..5, out_ch k); 3 h-shifted matmuls accumulate
    in PSUM => 6 output columns x 16 ch x 444 pixels per 3 matmuls
  - floor: v1 = y_int/256 - 255/512 (ACT), v2 = v1 + 1.5*2^23 (DVE, RNE
    leaves floor+C), PE-transpose back to [h', (w',k)], final
    out = v2/256 - 49152 fused into the PSUM->SBUF copy (DVE)
  - store [h', 3552] f32 rows contiguously (perfect DMA)
"""

import numpy as np

import concourse.mybir as mybir
from concourse import bass, tile
from concourse.masks import make_identity

N_CORES = 8
B_FULL = 32
B_CORE = B_FULL // N_CORES  # 4 images per core
H = W = 224
C = K = 16
HO = WO = 222

F32 = mybir.dt.float32
F16 = mybir.dt.float16

C_RND = 12582912.0  # 1.5 * 2**23: magic addend, RNE-to-integer for |v| < 2**22
INV_S = 1.0 / 256.0
FLOOR_BIAS = -255.0 / 512.0
OUT_BIAS = -49152.0  # -(C_RND / 256)

N_BLK = 37  # 37 blocks x 6 output w's = 222


def _split_multi_waits(nc):
    """The installed walrus only encodes ONE sync wait per instruction.
    Hoist extra waits onto NoOps inserted just before, same engine."""
    for f in nc.m.functions:
        for bb in f.blocks:
            new_list = []
            changed = False
            for ins in bb.instructions:
                si = ins.sync_info
                if si is not None and si.on_wait and len(si.on_wait) > 1:
                    waits = list(si.on_wait)
                    for wt in waits[:-1]:
                        nop = mybir.InstNoOp(
                            name=f"NOPW-{nc.next_id()}", ins=[], outs=[]
                        )
                        nop.engine = ins.engine
                        nop.sync_info = mybir.SyncInfo(on_wait=[wt], on_update=[])
                        new_list.append(nop)
                    ins.sync_info = mybir.SyncInfo(
                        on_wait=[waits[-1]], on_update=list(si.on_update or [])
                    )
                    changed = True
                new_list.append(ins)
            if changed:
                bb.instructions = new_list


def _build_nc(stage_limit: int = 7, reps: int = 1):
    # stage_limit: 1=loads 2=+quant 3=+in_transpose 4=+conv 5=+floor
    #              6=+out_transpose 7=+store (full kernel). reps>1 repeats
    #              the whole pipeline in-NEFF (timing harness only). The
    #              runner uses stage_limit=7, reps=1.
    nc = bass.Bass("TRN2", num_devices=N_CORES)
    x_d = nc.dram_tensor("x", [B_CORE * H, W * C], F32, kind="ExternalInput")
    wb_d = nc.dram_tensor("wb", [3, 128, 96], F16, kind="ExternalInput")
    y_d = nc.dram_tensor("y", [B_CORE, HO, WO * K], F32, kind="ExternalOutput")

    add = mybir.AluOpType.add
    mult = mybir.AluOpType.mult
    COPY = mybir.ActivationFunctionType.Copy

    htiles = ((0, 128), (128, 96))  # (row offset, rows) covering 224
    hchunks = ((0, 128), (128, 94))  # output h' chunks covering 222

    with tile.TileContext(nc) as tc:
        with (
            tc.tile_pool(name="consts", bufs=1) as consts,
            tc.tile_pool(name="xf", bufs=2) as xf_pool,
            tc.tile_pool(name="xq", bufs=1) as xq_pool,
            tc.tile_pool(name="xqt", bufs=4) as xqt_pool,
            tc.tile_pool(name="v", bufs=3) as v_pool,
            tc.tile_pool(name="st", bufs=1) as st_pool,
            tc.tile_pool(name="pst", bufs=3, space="PSUM") as ps_in_pool,
            tc.tile_pool(name="psy", bufs=2, space="PSUM") as ps_y_pool,
            tc.tile_pool(name="pso", bufs=3, space="PSUM") as ps_o_pool,
        ):
            ident16 = consts.tile([128, 128], F16, tag="id16")
            make_identity(nc, ident16[:])
            ident32 = consts.tile([128, 128], F32, tag="id32")
            make_identity(nc, ident32[:])
            wtiles = []
            for kh in range(3):
                wt = consts.tile([128, 96], F16, tag=f"w{kh}")
                nc.sync.dma_start(out=wt[:], in_=wb_d[kh])
                wtiles.append(wt)

            for pair in range(2 * reps):
                pair = pair % 2
                # ---- load + quantize both images of the pair ----
                xq_tiles = {}
                for ii in range(2):
                    img = 2 * pair + ii
                    for ht, (r0, pr) in enumerate(htiles):
                        xf = xf_pool.tile([128, W * C], F32, tag="xf")
                        nc.sync.dma_start(
                            out=xf[:pr, :],
                            in_=x_d[H * img + r0 : H * img + r0 + pr, :],
                        )
                        if stage_limit < 2:
                            continue
                        tt = xf_pool.tile([128, W * C], F32, tag="tt")
                        nc.scalar.activation(
                            out=tt[:pr, :], in_=xf[:pr, :], func=COPY,
                            bias=C_RND, scale=256.0,
                        )
                        xq = xq_pool.tile([128, W * C], F16, tag=f"xq{ii}{ht}")
                        nc.vector.tensor_scalar(
                            out=xq[:pr, :], in0=tt[:pr, :],
                            scalar1=-C_RND, scalar2=None, op0=add,
                        )
                        xq_tiles[(ii, ht)] = xq

                st_all = st_pool.tile(
                    [128, 2, 2, N_BLK * 96], F32, tag="st_all", name="st_all"
                )

                # ---- per w-block: transpose in, conv, floor, transpose out ----
                for blk in range(N_BLK):
                    if stage_limit < 3:
                        break
                    xqt = xqt_pool.tile([128, 2, 224], F16, tag="xqt")
                    pst = ps_in_pool.tile([128, 2, 224], F16, tag="pst")
                    for ii in range(2):
                        for ht, (r0, pr) in enumerate(htiles):
                            nc.tensor.transpose(
                                out=pst[:, ii, r0 : r0 + pr],
                                in_=xq_tiles[(ii, ht)][:pr, 96 * blk : 96 * blk + 128],
                                identity=ident16[:pr, :pr],
                            )
                    if blk % 2 == 0:
                        nc.vector.tensor_copy(out=xqt[:], in_=pst[:])
                    else:
                        nc.scalar.activation(out=xqt[:], in_=pst[:], func=COPY)

                    if stage_limit < 4:
                        continue
                    psy = ps_y_pool.tile([96, 2, WO], F32, tag="psy")
                    for s in range(3):
                        nc.tensor.matmul(
                            out=psy[:],
                            lhsT=wtiles[s][:],
                            rhs=xqt[:, :, s : s + WO],
                            start=(s == 0),
                            stop=(s == 2),
                        )

                    if stage_limit < 5:
                        continue
                    v1 = v_pool.tile([96, 2, WO], F32, tag="v1")
                    nc.scalar.activation(
                        out=v1[:], in_=psy[:], func=COPY,
                        bias=FLOOR_BIAS, scale=INV_S,
                    )
                    v2 = v_pool.tile([96, 2, WO], F32, tag="v2")
                    nc.vector.tensor_scalar(
                        out=v2[:], in0=v1[:], scalar1=C_RND, scalar2=None, op0=add,
                    )

                    if stage_limit < 6:
                        continue
                    pso = ps_o_pool.tile([128, 2, 2, 96], F32, tag="pso")
                    for ii in range(2):
                        for ch, (h0, hc) in enumerate(hchunks):
                            nc.tensor.transpose(
                                out=pso[:hc, ii, ch, :],
                                in_=v2[:, ii, h0 : h0 + hc],
                                identity=ident32[:96, :96],
                            )
                    nc.vector.tensor_scalar(
                        out=st_all[:, :, :, 96 * blk : 96 * blk + 96],
                        in0=pso[:],
                        scalar1=INV_S, scalar2=OUT_BIAS,
                        op0=mult, op1=add,
                    )
                    if blk == 18 and stage_limit >= 7:
                        for ii in range(2):
                            img = 2 * pair + ii
                            for ch, (h0, hc) in enumerate(hchunks):
                                nc.sync.dma_start(
                                    out=y_d[img, h0 : h0 + hc, : 19 * 96],
                                    in_=st_all[:hc, ii, ch, : 19 * 96],
                                )

                # ---- store (blocks 19..36; 0..18 streamed at blk 18) ----
                if stage_limit < 7:
                    continue
                for ii in range(2):
                    img = 2 * pair + ii
                    for ch, (h0, hc) in enumerate(hchunks):
                        nc.sync.dma_start(
                            out=y_d[img, h0 : h0 + hc, 19 * 96 :],
                            in_=st_all[:hc, ii, ch, 19 * 96 :],
                        )

    _split_multi_waits(nc)
    return nc


def _banded_weights(w: np.ndarray) -> np.ndarray:
    """w [3,3,16,16] f32 -> wb [3, 128, 96] fp16 banded lhsT matrices.

    wb[kh][16*dw + c, 16*j + k] = round(w*256)[kh, dw - j, c, k]
    for 0 <= dw - j <= 2, j in 0..5."""
    wq = np.round(w.astype(np.float32) * np.float32(256.0))  # RNE, exact
    assert np.abs(wq).max() < 240, "w_int exceeds fp16-exact budget"
    wb = np.zeros((3, 128, 96), dtype=np.float32)
    for kh in range(3):
        for j in range(6):
            for kw in range(3):
                dw = j + kw
                wb[kh, 16 * dw : 16 * dw + 16, 16 * j : 16 * j + 16] = wq[kh, kw]
    return wb.astype(np.float16)


_RUNNER = None


def _get_runner():
    """Build the Bass program once and return a callable(in_maps)->results."""
    global _RUNNER
    if _RUNNER is None:
        _RUNNER = _make_runner(_build_nc())
    return _RUNNER


def _make_runner(nc):
    """Mirrors concourse.bass2jax.run_bass_via_pjrt's multi-core path but
    caches the jitted executable so repeated calls don't recompile."""
    import jax
    from jax.sharding import Mesh, PartitionSpec
    from jax.experimental.shard_map import shard_map
    from concourse.bass2jax import (
        _bass_exec_p,
        install_neuronx_cc_hook,
        partition_id_tensor,
    )

    install_neuronx_cc_hook()

    partition_name = nc.partition_id_tensor.name if nc.partition_id_tensor else None
    in_names, out_names, out_avals, zero_outs = [], [], [], []
    for alloc in nc.m.functions[0].allocations:
        if not isinstance(alloc, mybir.MemoryLocationSet):
            continue
        name = alloc.memorylocations[0].name
        if alloc.kind == "ExternalInput":
            if name != partition_name:
                in_names.append(name)
        elif alloc.kind == "ExternalOutput":
            out_names.append(name)
            shape = tuple(alloc.tensor_shape)
            dtype = mybir.dt.np(alloc.dtype)
            out_avals.append(jax.core.ShapedArray(shape, dtype))
            zero_outs.append(np.zeros(shape, dtype))
    n_params = len(in_names)
    n_outs = len(out_avals)
    all_in_names = list(in_names) + list(out_names)
    if partition_name is not None:
        all_in_names.append(partition_name)

    def _body(*args):
        operands = list(args)
        if partition_name is not None:
            operands.append(partition_id_tensor())
        outs = _bass_exec_p.bind(
            *operands,
            out_avals=tuple(out_avals),
            in_names=tuple(all_in_names),
            out_names=tuple(out_names),
            lowering_input_output_aliases=(),
            sim_require_finite=True,
            sim_require_nnan=True,
            nc=nc,
        )
        return tuple(outs)

    devices = jax.devices()[:N_CORES]
    assert len(devices) == N_CORES, f"need {N_CORES} devices, got {len(devices)}"
    mesh = Mesh(np.asarray(devices), ("core",))
    in_specs = (PartitionSpec("core"),) * (n_params + n_outs)
    out_specs = (PartitionSpec("core"),) * n_outs
    sharded = jax.jit(
        shard_map(_body, mesh=mesh, in_specs=in_specs, out_specs=out_specs,
                  check_rep=False),
        donate_argnums=tuple(range(n_params, n_params + n_outs)),
        keep_unused=True,
    )

    state = {
        "sharded": sharded,
        "in_names": in_names,
        "out_names": out_names,
        "out_avals": out_avals,
        "zero_outs": zero_outs,
        "n_cores": N_CORES,
    }

    def runner(in_maps):
        per_core = [[np.asarray(m[nm]) for nm in in_names] for m in in_maps]
        concat_in = [
            np.concatenate([per_core[c][i] for c in range(N_CORES)], axis=0)
            for i in range(n_params)
        ]
        concat_zeros = [
            np.zeros((N_CORES * z.shape[0], *z.shape[1:]), z.dtype)
            for z in zero_outs
        ]
        out_arrs = state["sharded"](*concat_in, *concat_zeros)
        return [
            {
                nm: np.asarray(out_arrs[i]).reshape(
                    N_CORES, *out_avals[i].shape
                )[c]
                for i, nm in enumerate(out_names)
            }
            for c in range(N_CORES)
        ]

    runner.state = state
    return runner


def kernel(x: np.ndarray, w: np.ndarray, fixed_point) -> np.ndarray:
    assert int(fixed_point) == 8, f"kernel hardcodes fixed_point=8, got {fixed_point}"
    x = np.ascontiguousarray(np.asarray(x, dtype=np.float32))
    assert x.shape == (B_FULL, H, W, C), x.shape
    assert np.abs(x).max() * 256.0 < 2040.0, "x_int exceeds fp16-exact budget"

    wb = _banded_weights(np.asarray(w, dtype=np.float32))
    runner = _get_runner()

    in_maps = []
    for core in range(N_CORES):
        xs = x[B_CORE * core : B_CORE * (core + 1)].reshape(B_CORE * H, W * C)
        in_maps.append({"x": xs, "wb": wb})

    results = runner(in_maps)
    out = np.concatenate(
        [r["y"].reshape(B_CORE, HO, WO, K) for r in results], axis=0
    )
    return out
